# revision 17
# baseline (speedup 1.0000x reference)
"""ConvEnc (conv3x3 + BN + LIF(T=4) firing rate) — Trainium2 Bass kernel.

Math: with input constant across T timesteps, the LIF firing rate is a
piecewise-constant step function of the conv+BN output u with at most T
thresholds.  Exact fp32 thresholds are found host-side by bit-bisection
of the fp32-faithful recurrence; the per-channel BN affine (monotone,
inv>0) is folded into per-channel thresholds on the *raw* conv output.
The conv (Cin=1, 3x3 SAME) is a K=9 im2col matmul on the tensor engine.

Post-conv, ONE custom DVE instruction per PSUM tile computes the 2-bit
threshold code  k = (c>=t1)+(c>=t2)+(c>=t3) in {0,1,2,3}; vector
mult-adds pack 4 adjacent pixels into one byte (base-4 digits), written
uint8, with per-image-row accum sums feeding a row-fired flag output.
The device output is 2 bits/pixel (16x smaller than fp32), which matters
because the axon tunnel to the device moves ~40-90 MB/s with no
compression and ~100ms fixed latency per host fetch.

Launch path: the stock run_bass_kernel_spmd re-jits (retrace + relower +
NEFF-cache lookup) and uploads donated zero output buffers on every
call; this kernel instead builds one jax.jit(shard_map(bass_exec)) at
first use and caches it module-level, along with device-resident copies
of the (hashed) inputs.  A chained jit'd on-device compaction then
extracts just the nonzero 32-byte output rows (LIF spikes are sparse),
so a warm call costs one host fetch of ~1.7MB instead of 16.7MB packed
(or 268MB fp32); the fp32 firing rate is reconstructed host-side into a
calloc'd buffer by a 256-entry LUT scatter.

Sharding: data-parallel over batch N across 8 NeuronCores; weights /
thresholds replicated; no collectives.
"""
import hashlib
import numpy as np
from contextlib import ExitStack

import jax
from jax.experimental.shard_map import shard_map
from jax.sharding import Mesh, NamedSharding, PartitionSpec

import concourse.bass as bass
import concourse.bacc as bacc
import concourse.tile as tile
from concourse import mybir, bass2jax
from concourse.bass_utils import run_bass_kernel_spmd

F32 = mybir.dt.float32
U8 = mybir.dt.uint8
N_CORES = 8
H = W = 128
C = 128
HW = H * W
PACKED = HW // 4    # packed bytes per channel per image
PADW = 132          # padded image row stride (130 cols used)
ROWS_PER_RHS = 32   # rhs tile rows; keeps matmul rhs AP offsets < 16 KiB


# ---------------- host-side threshold math (exact fp32) -------------------
def _lif_spike_count_f32(u, T, tau):
    u = np.asarray(u, np.float32)
    v = np.zeros_like(u)
    n = np.zeros_like(u)
    inv_tau = np.float32(1.0) / np.float32(tau)
    one = np.float32(1.0)
    for _ in range(T):
        t = (u - v).astype(np.float32)
        h = (v + (t * inv_tau).astype(np.float32)).astype(np.float32)
        s = ((h - one).astype(np.float32) >= 0).astype(np.float32)
        v = (h * (one - s)).astype(np.float32)
        n = n + s
    return n


def _bisect_f32(pred, lo, hi):
    assert lo > 0 and hi > 0 and not pred(lo) and pred(hi)
    ilo = int(np.float32(lo).view(np.int32))
    ihi = int(np.float32(hi).view(np.int32))
    while ihi - ilo > 1:
        imid = (ilo + ihi) // 2
        mid = np.int32(imid).view(np.float32)
        if pred(mid):
            ihi = imid
        else:
            ilo = imid
    return np.int32(ihi).view(np.float32)


_U_THR_CACHE = {}


def _lif_u_thresholds(T, tau):
    key = (T, tau)
    if key in _U_THR_CACHE:
        return _U_THR_CACHE[key]
    us = np.linspace(0.0, 8.0, 4_000_001, dtype=np.float32)
    ns = _lif_spike_count_f32(us, T, tau)
    assert np.all(np.diff(ns) >= 0), "LIF spike count not monotone"
    levels = np.unique(ns)
    assert levels[0] == 0
    thr, counts = [], []
    for lv in levels[1:]:
        thr.append(_bisect_f32(
            lambda x: _lif_spike_count_f32(x, T, tau) >= lv,
            np.float32(2**-20), np.float32(16.0)))
        counts.append(float(lv))
    w = np.diff([0.0] + counts)
    r = (np.array(thr, np.float32), w.astype(np.float32))
    _U_THR_CACHE[key] = r
    return r


_CH_THR_CACHE = {}


def _channel_thresholds(u_thr, inv, bias_term):
    key = (u_thr.tobytes(), inv.tobytes(), bias_term.tobytes())
    hit = _CH_THR_CACHE.get(key)
    if hit is not None:
        return hit
    assert np.all(inv > 0), "negative BN scale not supported"
    nch = inv.shape[0]
    out = np.empty((len(u_thr), nch), np.float32)
    for j, u in enumerate(u_thr):
        for p in range(nch):
            iv, b = np.float32(inv[p]), np.float32(bias_term[p])
            pred = lambda cc: np.float32(np.float32(cc * iv) + b) >= u
            out[j, p] = _bisect_f32(pred, np.float32(2**-20), np.float32(64.0))
    _CH_THR_CACHE.clear()
    _CH_THR_CACHE[key] = out
    return out


# ---------------- custom DVE op: 2-bit threshold code ---------------------
_CODE_OP = None


def _get_code_op():
    global _CODE_OP
    if _CODE_OP is not None:
        return _CODE_OP
    from concourse.dve_spec import Spec, Src0, Src1, C0, C1, Latch, lower
    from concourse.dve_uop import DveOpSpec
    import concourse.dve_ops as dve_ops

    s1 = (Src0 >= C0)
    s2 = (Src0 >= C1)
    s3 = (Src0 >= Latch(Src1))
    body = (s1 + s2) + s3

    def ref(in0, in1, s0, s1v, imm2):
        r = ((in0 >= s0).astype(np.float32)
             + (in0 >= s1v).astype(np.float32)
             + (in0 >= in1).astype(np.float32))
        return r.astype(np.float32)

    spec = Spec(body=body, reference=ref)
    name = "LIF_CODE4_ANT"
    if name in dve_ops._SUB_OPCODE_FOR_NAME:
        _CODE_OP = next(o for o in dve_ops.OPS if o.name == name)
        return _CODE_OP
    row = dve_ops._CUSTOM_DVE_ROW_BASE + len(dve_ops.OPS)
    shas = {}
    for ver in ("v3", "v4"):
        shas[ver] = DveOpSpec(name=name, opcode=row,
                              uops=lower(spec, ver=ver), rd1_en=True).sha(ver)
    op = dve_ops.DveOp(name, spec, subdim=False, uops_sha=shas)
    dve_ops.OPS.append(op)
    dve_ops._SUB_OPCODE_FOR_NAME[name] = row
    dve_ops.CUSTOM_DVE_SPECS[name] = spec
    _CODE_OP = op
    return op


# ---------------- bass program (SPMD over 8 cores) ------------------------
_NC_CACHE = {}


def _build_nc(n_per, psum_free=2048):
    key = (n_per, psum_free)
    if key in _NC_CACHE:
        return _NC_CACHE[key]
    nc = bacc.Bacc("TRN2", target_bir_lowering=False, debug=False,
                   num_devices=N_CORES)
    xp = nc.declare_dram_parameter("xp", [n_per, H + 2, PADW], F32,
                                   isOutput=False)
    w2 = nc.declare_dram_parameter("w2", [32, C], F32, isOutput=False)
    th1 = nc.declare_dram_parameter("th1", [C, 1], F32, isOutput=False)
    th2 = nc.declare_dram_parameter("th2", [C, 1], F32, isOutput=False)
    th3 = nc.declare_dram_parameter("th3", [C, 1], F32, isOutput=False)
    out = nc.declare_dram_parameter("out", [n_per, C, PACKED], U8,
                                    isOutput=True)
    flg = nc.declare_dram_parameter("flg", [n_per, C, H], U8, isOutput=True)
    code_op = _get_code_op()
    rows_per_psum = psum_free // W            # 16
    halves = ROWS_PER_RHS // rows_per_psum    # 2 psum tiles per quad
    pk = psum_free // 4                       # packed bytes per psum tile
    wb = W // 4                               # packed bytes per image row

    mult = mybir.AluOpType.mult
    add = mybir.AluOpType.add
    is_gt = mybir.AluOpType.is_gt

    with ExitStack() as ctx:
        tc = ctx.enter_context(tile.TileContext(nc))
        const = ctx.enter_context(tc.tile_pool(name="const", bufs=1))
        rhs_p = ctx.enter_context(tc.tile_pool(name="rhs", bufs=2))
        ps_p = ctx.enter_context(tc.tile_pool(name="ps", bufs=2, space="PSUM"))
        ct_p = ctx.enter_context(tc.tile_pool(name="ct", bufs=2))
        tm_p = ctx.enter_context(tc.tile_pool(name="tm", bufs=2))
        out_p = ctx.enter_context(tc.tile_pool(name="outp", bufs=3))
        rs_p = ctx.enter_context(tc.tile_pool(name="rs", bufs=2))
        fl_p = ctx.enter_context(tc.tile_pool(name="fl", bufs=2))

        w2_s = const.tile([32, C], F32)
        nc.sync.dma_start(w2_s[:], w2[:])
        t_s = []
        for j, th in enumerate((th1, th2, th3)):
            t = const.tile([C, 1], F32, tag=f"thr{j}")
            nc.sync.dma_start(t[:], th[:])
            t_s.append(t)

        # One-time zero of both rhs SBUF slots: the PE contracts the full
        # 32-row group, so K-pad rows 9..31 must be finite (weights there are
        # zero).  Those rows are never rewritten, so the zeros persist.
        for _ in range(2):
            st = rhs_p.tile([32, ROWS_PER_RHS, W], F32, tag="rhs")
            nc.gpsimd.memset(st[:], 0.0)

        for n in range(n_per):
            rs = rs_p.tile([C, H], F32, tag="rs")
            for quad in range(H // ROWS_PER_RHS):
                y0 = quad * ROWS_PER_RHS
                rhs_t = rhs_p.tile([32, ROWS_PER_RHS, W], F32, tag="rhs")
                for k in range(9):
                    dy, dx = k // 3, k % 3
                    nc.sync.dma_start(
                        rhs_t[k:k + 1],
                        xp[n:n + 1, y0 + dy:y0 + dy + ROWS_PER_RHS,
                           dx:dx + W])
                po = out_p.tile([C, halves, pk], U8, tag="po")
                for b in range(halves):
                    ps = ps_p.tile([C, psum_free], F32, tag="ps")
                    for m in range(rows_per_psum // 4):
                        rr = b * rows_per_psum + m * 4
                        nc.tensor.matmul(
                            ps[:, m * 512:(m + 1) * 512], w2_s[:],
                            rhs_t[:, rr:rr + 4, :],
                            start=True, stop=True)
                    # 2-bit code per pixel: k = (c>=t1)+(c>=t2)+(c>=t3)
                    ct = ct_p.tile([C, pk, 4], F32, tag="ct")
                    nc.vector._custom_dve(
                        code_op, out=ct[:], in0=ps[:], in1=t_s[2][:],
                        s0=t_s[0][:], s1=t_s[1][:], imm2=0.0)
                    # pack 4 adjacent pixels into one byte (base-4 digits);
                    # one instr per image row so accum_out yields per-row
                    # spike sums for the sparsity flags
                    t01 = tm_p.tile([C, pk], F32, tag="t01")
                    t23 = tm_p.tile([C, pk], F32, tag="t23")
                    nc.vector.scalar_tensor_tensor(
                        t01[:], ct[:, :, 1:2], 4.0, ct[:, :, 0:1], mult, add)
                    nc.vector.scalar_tensor_tensor(
                        t23[:], ct[:, :, 3:4], 4.0, ct[:, :, 2:3], mult, add)
                    for r in range(rows_per_psum):
                        row = y0 + b * rows_per_psum + r
                        nc.vector.scalar_tensor_tensor(
                            po[:, b:b + 1, r * wb:(r + 1) * wb],
                            t23[:, r * wb:(r + 1) * wb], 16.0,
                            t01[:, r * wb:(r + 1) * wb], mult, add,
                            accum_out=rs[:, row:row + 1])
                p0 = y0 * wb
                nc.sync.dma_start(
                    out[n, :, p0:p0 + halves * pk], po[:])
            fl = fl_p.tile([C, H], U8, tag="fl")
            nc.vector.tensor_scalar(fl[:], rs[:], 0.0, None, is_gt)
            nc.sync.dma_start(flg[n], fl[:])
    nc.compile()
    _NC_CACHE[key] = nc
    return nc


# ---------------- cached PJRT runner --------------------------------------
_RUNNER = {}


def _get_runner(n_per):
    if n_per in _RUNNER:
        return _RUNNER[n_per]
    nc = _build_nc(n_per)
    bass2jax.install_neuronx_cc_hook()
    pid_name = nc.partition_id_tensor.name if nc.partition_id_tensor else None
    in_names, out_names, out_avals = [], [], []
    for alloc in nc.m.functions[0].allocations:
        if not isinstance(alloc, mybir.MemoryLocationSet):
            continue
        name = alloc.memorylocations[0].name
        if alloc.kind == "ExternalInput" and name != pid_name:
            in_names.append(name)
        elif alloc.kind == "ExternalOutput":
            out_names.append(name)
            out_avals.append(jax.core.ShapedArray(
                tuple(alloc.tensor_shape), mybir.dt.np(alloc.dtype)))
    bind_names = tuple(in_names) + ((pid_name,) if pid_name else ())

    def _body(*args):
        operands = list(args)
        if pid_name:
            operands.append(bass2jax.partition_id_tensor())
        outs = bass2jax._bass_exec_p.bind(
            *operands, out_avals=tuple(out_avals), in_names=bind_names,
            out_names=tuple(out_names), lowering_input_output_aliases=(),
            sim_require_finite=True, sim_require_nnan=True, nc=nc)
        return tuple(outs)

    mesh = Mesh(np.asarray(jax.devices()[:N_CORES]), ("core",))
    fn = jax.jit(
        shard_map(_body, mesh=mesh,
                  in_specs=(PartitionSpec("core"),) * len(in_names),
                  out_specs=(PartitionSpec("core"),) * len(out_names),
                  check_rep=False),
        keep_unused=True)
    r = dict(nc=nc, fn=fn, in_names=in_names, out_names=out_names,
             mesh=mesh, sharding=NamedSharding(mesh, PartitionSpec("core")),
             dev={}, gather={}, no_gather=False)
    _RUNNER[n_per] = r
    return r


def _get_gather_fn(R, n_per, K):
    """jit'd on-device gather of K 32-byte rows per core from the
    device-resident packed output (avoids downloading the dense buffer)."""
    if K in R["gather"]:
        return R["gather"][K]
    import jax.numpy as jnp
    rows_total = n_per * C * H

    def _body(buf, idx):
        rows = buf.reshape(rows_total, W // 4)
        return jnp.take(rows, idx, axis=0, mode="clip")

    fn = jax.jit(
        shard_map(_body, mesh=R["mesh"],
                  in_specs=(PartitionSpec("core"), PartitionSpec("core")),
                  out_specs=PartitionSpec("core"), check_rep=False))
    R["gather"][K] = fn
    return fn


def _get_compact_fn(R, n_per, K):
    """jit'd on-device row compaction: gather the first K nonzero 32-byte
    rows of the packed output and emit them as one [K, 32+nb] u8 array
    per core (32 data bytes + the row id as nb LE bytes).  The j-th nonzero
    row is found with a branchless lower_bound over the cumsum of the
    row-nonzero flags — gather/where/shift only; neuronx-cc rejects
    sort and crashes on scatter/concatenate here.  Chained after the
    bass_exec dispatch this costs ONE host sync for the whole call
    (per-fetch latency dominates the wire time on this tunnel)."""
    key = ("compact", K)
    if key in R["gather"]:
        return R["gather"][key]
    import jax.numpy as jnp
    rows_total = n_per * C * H
    wb = W // 4
    nb = 2 if rows_total <= 65536 else 4        # id bytes per row
    span = 1 << (rows_total - 1).bit_length()   # pow2 search span

    def _body(buf):
        rows = buf.reshape(rows_total, wb)
        f = jnp.max(rows, axis=1) > 0
        pos = jnp.cumsum(f.astype(jnp.int32))
        tgt = jnp.arange(1, K + 1, dtype=jnp.int32)
        idx = jnp.zeros((K,), jnp.int32)
        s = span // 2
        while s >= 1:
            cand = idx + s
            v = jnp.take(pos, jnp.minimum(cand, rows_total) - 1,
                         mode="clip")
            idx = jnp.where(v < tgt, cand, idx)
            s //= 2
        idx = jnp.minimum(idx, rows_total - 1)
        out_rows = jnp.take(rows, idx, axis=0, mode="clip")
        col = jnp.arange(wb + nb, dtype=jnp.int32)[None, :]
        rowv = out_rows[:, jnp.minimum(jnp.arange(wb + nb), wb - 1)]
        sh = jnp.maximum(col - wb, 0) * 8
        idv = ((idx[:, None] >> sh) & 255).astype(jnp.uint8)
        return jnp.where(col < wb, rowv, idv)

    fn = jax.jit(
        shard_map(_body, mesh=R["mesh"],
                  in_specs=(PartitionSpec("core"),),
                  out_specs=PartitionSpec("core"), check_rep=False))
    R["gather"][key] = fn
    return fn


def _digest(*arrs):
    h = hashlib.blake2b(digest_size=16)
    for a in arrs:
        h.update(np.ascontiguousarray(a).tobytes())
    return h.digest()


# code k -> firing rate; byte -> 4 pixels' rates
def _make_lut(code_fr):
    lut = np.empty((256, 4), np.float32)
    b = np.arange(256, dtype=np.uint32)
    for k in range(4):
        lut[:, k] = code_fr[(b >> (2 * k)) & 3]
    return lut


# ---------------- public entry point --------------------------------------
_FALLBACK = {"on": False}


def kernel(x, conv_w, gamma, beta, running_mean, running_var, T, tau=2.0,
           **_unused):
    x = np.asarray(x, np.float32)
    conv_w = np.asarray(conv_w, np.float32)
    gamma = np.asarray(gamma, np.float32)
    beta = np.asarray(beta, np.float32)
    running_mean = np.asarray(running_mean, np.float32)
    running_var = np.asarray(running_var, np.float32)
    T = int(T)
    tau = float(tau)
    N = x.shape[0]
    assert x.shape == (N, 1, H, W) and conv_w.shape == (C, 1, 3, 3)
    assert N % N_CORES == 0
    n_per = N // N_CORES

    inv = (gamma * (1.0 / np.sqrt(running_var + np.float32(1e-5),
                                  dtype=np.float32)).astype(np.float32)
           ).astype(np.float32)
    bias_term = (beta - running_mean * inv).astype(np.float32)
    u_thr, u_w = _lif_u_thresholds(T, tau)
    assert len(u_thr) == 3, \
        "kernel hardcodes the 3-threshold (T=4/tau=2) structure"
    t = _channel_thresholds(u_thr, inv, bias_term)
    code_fr = np.concatenate(
        [[0.0], np.cumsum(u_w)]).astype(np.float32) / np.float32(T)
    lut = _make_lut(code_fr)

    xpad = np.zeros((N, H + 2, PADW), np.float32)
    xpad[:, 1:H + 1, 1:W + 1] = x[:, 0]
    w2 = np.zeros((32, C), np.float32)
    w2[:9] = conv_w[:, 0].reshape(C, 9).T
    th = [np.ascontiguousarray(t[j][:, None]) for j in range(3)]

    rows_total = n_per * C * H
    wb = W // 4

    if not _FALLBACK["on"]:
        try:
            R = _get_runner(n_per)
            dev = R["dev"]
            hx = _digest(xpad)
            if dev.get("hx") != hx:
                dev["xp"] = jax.device_put(xpad, R["sharding"])
                dev["hx"] = hx
            hc = _digest(w2, *th)
            if dev.get("hc") != hc:
                g = [np.concatenate([a] * N_CORES, axis=0)
                     for a in (w2, *th)]
                dev["consts"] = [jax.device_put(a, R["sharding"]) for a in g]
                dev["hc"] = hc
            args = {"xp": dev["xp"], "w2": dev["consts"][0],
                    "th1": dev["consts"][1], "th2": dev["consts"][2],
                    "th3": dev["consts"][3]}
            outs = R["fn"](*[args[nm] for nm in R["in_names"]])
            packed_dev = outs[R["out_names"].index("out")]
            flg_dev = outs[R["out_names"].index("flg")]

            # Single-sync sparse fetch: chain an on-device compaction
            # (nonzero rows + ids + count as one payload) after the bass
            # dispatch — dispatches pipeline, so the whole call costs one
            # ~100ms host fetch plus the (small) payload bytes.
            if not R.get("no_compact"):
                try:
                    K = R.get("K", 6144)
                    nb = 2 if rows_total <= 65536 else 4
                    idt = np.uint16 if nb == 2 else np.int32
                    payload = np.asarray(
                        _get_compact_fn(R, n_per, K)(packed_dev)
                    ).reshape(N_CORES, K, wb + nb)
                    overflow = False
                    decoded = []
                    for c in range(N_CORES):
                        rows = payload[c, :, :wb]
                        rid = np.ascontiguousarray(
                            payload[c, :, wb:]).view(idt).ravel() \
                            .astype(np.int32)
                        # valid prefix: row ids strictly increase and the
                        # gathered rows are nonzero; cnt==K is ambiguous
                        # with capacity overflow, so treat it as overflow
                        d = np.diff(rid)
                        m1 = K if (d > 0).all() else int(np.argmin(d > 0)) + 1
                        z = rows.max(axis=1) == 0
                        m2 = K if not z.any() else int(np.argmax(z))
                        cnt = min(m1, m2)
                        if cnt >= K:
                            overflow = True
                            break
                        decoded.append((rid, rows, cnt))
                    if overflow:
                        # take the dense hit this call; grow K once (compile
                        # cost of the unrolled lower_bound blows up with K),
                        # then give up on compaction for this process
                        if K < 16384:
                            R["K"] = 16384
                        else:
                            R["no_compact"] = True
                    else:
                        fr4 = np.zeros((N * C * PACKED, 4), np.float32)
                        for c, (rid, rows, cnt) in enumerate(decoded):
                            if cnt == 0:
                                continue
                            rows = rows[:cnt]
                            ri, ci = np.nonzero(rows)
                            if ri.size:
                                gb = ((c * rows_total
                                       + rid[:cnt][ri].astype(np.int64))
                                      * wb + ci)
                                fr4[gb] = lut[rows[ri, ci]]
                        return fr4.reshape(N, C, H, W)
                except Exception:
                    import traceback
                    traceback.print_exc()
                    R["no_compact"] = True

            # Two-sync sparse fetch: download per-row spike flags (tiny),
            # gather only the rows that fired on-device, download those.
            if not R["no_gather"]:
                try:
                    flags = np.asarray(flg_dev).reshape(N_CORES, rows_total)
                    nz_rows = [np.flatnonzero(flags[c])
                               for c in range(N_CORES)]
                    kmax = max(r_.size for r_ in nz_rows)
                    if kmax == 0:
                        return np.zeros((N, C, H, W), np.float32)
                    if kmax <= rows_total // 4:
                        K = max(1024, 1 << (kmax - 1).bit_length())
                        idx = np.zeros(N_CORES * K, np.int32)
                        for c, r_ in enumerate(nz_rows):
                            idx[c * K:c * K + r_.size] = r_
                        idx_dev = jax.device_put(idx, R["sharding"])
                        rows_np = np.asarray(
                            _get_gather_fn(R, n_per, K)(packed_dev, idx_dev))
                        fr4 = np.zeros((N * C * PACKED, 4), np.float32)
                        for c, r_ in enumerate(nz_rows):
                            data = rows_np[c * K:c * K + r_.size]
                            ri, ci = np.nonzero(data)
                            if ri.size:
                                gb = ((c * rows_total + r_[ri]) * wb + ci)
                                fr4[gb] = lut[data[ri, ci]]
                        return fr4.reshape(N, C, H, W)
                except Exception:
                    import traceback
                    traceback.print_exc()
                    R["no_gather"] = True

            packed = np.asarray(packed_dev).reshape(N, C, PACKED)
        except Exception:
            import traceback
            traceback.print_exc()
            _FALLBACK["on"] = True
            packed = None
    else:
        packed = None

    if packed is None:
        # robust path: stock SPMD launcher (re-jits each call, but still
        # uses the packed-output device kernel)
        nc = _build_nc(n_per)
        in_maps = [{"xp": xpad[c * n_per:(c + 1) * n_per], "w2": w2,
                    "th1": th[0], "th2": th[1], "th3": th[2]}
                   for c in range(N_CORES)]
        r = run_bass_kernel_spmd(nc, in_maps, list(range(N_CORES)))
        packed = np.concatenate(
            [r.results[c]["out"] for c in range(N_CORES)],
            axis=0).reshape(N, C, PACKED)

    # Dense reconstruction.  lut[0] == 0, so zero bytes map to 0.0 rows —
    # when the packed stream is sparse, calloc + scatter of the nonzero
    # bytes beats a full 268MB gather.
    flat = packed.reshape(-1)
    nz = np.flatnonzero(flat)
    if nz.size <= flat.size // 16:
        fr4 = np.zeros((flat.size, 4), np.float32)
        if nz.size:
            fr4[nz] = lut[flat[nz]]
        fr = fr4.reshape(N, C, H, W)
    else:
        fr = lut[flat].reshape(N, C, H, W)
    return fr
